# revision 4
# baseline (speedup 1.0000x reference)
""" Causal multi-head attention (B=4, S=2048, E=1024, H=16) on 8 trn2 NeuronCores.

Sharding: core c = (batch b = c//2, head-group g = c%2). Each core computes
attention for one batch element and 8 of the 16 heads, plus the partial
out-projection restricted to its heads' columns. Host sums the two partial
outputs per batch element and adds the out-projection bias.

v2 over the baseline:
  - fp16 operands everywhere (PSUM accum stays fp32): halves SBUF/DMA/DVE
    cost and lifts the fp32r moving>=256 constraint so diagonal blocks trim
    to exactly the causal band.
  - cross-iteration software pipelining: the next iteration's first q-window
    QKV runs as PE filler during the last attention window, writing
    dedicated qT0/kT0/v0 buffers (no WAR hazard with the current iteration);
    cheap DVE copies publish kT0/v0 into the main kT/v banks during window 0.
    The qb3 out-projection likewise slides into the next iteration's window 0.
    This keeps the PE instruction stream gapless across the loop back-edge
    (PE pstate drops to 1.2 GHz after any idle and needs 3us busy to
    recover 2.4 GHz, so gaps cost ~2x on every matmul that follows).

Dataflow (all "transposed" so no on-device transposes):
  qT, kT  [ch, s]   from  W_chunk @ x.T        (lhsT = W.T tiles, rhs = x.T)
  v       [s, ch]   from  x @ Wv.T             (lhsT = x.T tiles, rhs = Wv.T)
  scoresT [k, q]    from  lhsT = kT, rhs = qT  (per head, K = 64)
  ctxT    [d, q]    from  lhsT = v (+ones col), rhs = exp(scoresT)
  outP    [s, o]    from  lhsT = ctxT, rhs = Wo.T
Softmax without max subtraction (scores bounded ~|2|); normalizer from a
ones-column in v; causal masking via one multiplicative [128,128] triangular
mask tile applied to the exp'd diagonal blocks.
"""

import sys

sys.path.insert(0, "/opt/trn_rl_repo")

import numpy as np

import concourse.bass as bass  # noqa: F401  (registers engine classes)
import concourse.mybir as mybir
import concourse.tile as tile
from concourse import bacc
from concourse.bass_utils import run_bass_kernel_spmd

F32 = mybir.dt.float32
F8 = mybir.dt.float8e4
NP8 = None  # set below
F16 = mybir.dt.float16
NP16 = np.float16
NP8 = mybir.dt.np(mybir.dt.float8e4)
AF = mybir.ActivationFunctionType

B, S, E = 4, 2048, 1024
H, HD = 16, 64
GH = 8                 # heads handled per core
GC = GH * HD           # 512 channels per head-group
P = 128
NCORES = 8
NJ_ALL = S // P        # 16 k-blocks of 128
QB = S // 512          # 4 q-windows of 512

_program = {}


def _emit(tc, nc, xT, xT8, wqkT, wvT, woT, bqk, bv, out, bench_iters=0,
          has_bias=True):
    ctxmgr = []

    def pool(**kw):
        p = tc.tile_pool(**kw)
        ctxmgr.append(p)
        return p.__enter__()

    const = pool(name="const", bufs=1)
    kvp = pool(name="kv", bufs=1)
    xp = pool(name="xs", bufs=2)
    xp8 = pool(name="xs8", bufs=2)
    qp = pool(name="qt", bufs=2)
    cxp = pool(name="ctx", bufs=2)
    ep = pool(name="expt", bufs=6)
    osb = pool(name="osb", bufs=2)
    bp = pool(name="bcast", bufs=2)
    ps_s = pool(name="ps_s", bufs=2, space="PSUM")
    ps_m = pool(name="ps_m", bufs=4, space="PSUM")

    # ---- constants ----
    # DMA order matters at startup: the first qkT matmuls need wqk + the
    # first x strip; wo is only needed later, so it is emitted last.
    bqk_sb = const.tile([P, 8], F32)
    nc.sync.dma_start(bqk_sb[:], bqk.rearrange("c p -> p c"))
    bv_sb = const.tile([P, 4], F32)
    nc.sync.dma_start(bv_sb[:], bv.rearrange("c p -> p c"))
    wqk_sb = const.tile([P, 4, 2, 2 * GC], F8)    # [p, e4, i2, ch]
    wqk_r = wqkT.rearrange("(eo p) c -> p eo c", p=P)
    for e in range(8):
        eng = nc.sync if e % 2 == 0 else nc.gpsimd
        eng.dma_start(wqk_sb[:, e // 2, e % 2, :], wqk_r[:, e, :])
    wv_sb = const.tile([P, 8, GC], F16)
    wv_r = wvT.rearrange("(eo p) c -> p eo c", p=P)
    for e in range(8):
        eng = nc.gpsimd if e % 2 == 0 else nc.sync
        eng.dma_start(wv_sb[:, e, :], wv_r[:, e, :])
    wo_sb = const.tile([P, 4, E], F16)
    wo_r = woT.rearrange("(co p) o -> p co o", p=P)
    for co in range(4):
        nc.gpsimd.dma_start(wo_sb[:, co, :], wo_r[:, co, :])

    # Causal mask for diagonal 128-blocks: mask[p, u] = 1 if u >= p else 0
    # (p = key within block, u = query within block).
    tmpp = tc.tile_pool(name="tmpf", bufs=1)
    tmp = tmpp.__enter__()
    mask_f = tmp.tile([P, P], F32)
    nc.gpsimd.memset(mask_f[:], 1.0)
    nc.gpsimd.affine_select(
        out=mask_f[:],
        in_=mask_f[:],
        compare_op=mybir.AluOpType.is_ge,
        fill=0.0,
        base=0,
        pattern=[[1, P]],        # + u
        channel_multiplier=-1,   # - p   => keep where u - p >= 0
    )
    mask = const.tile([P, P], F16)
    nc.vector.tensor_copy(mask[:], mask_f[:])

    # ---- persistent tensors ----
    # kT holds k-blocks 4..15 (cols 512:2048); blocks 0..3 live in kT0/the
    # per-iteration copy, see below.
    kT_sb = kvp.tile([P, 4, S], F16)             # [p, c, s]; ch = c*128+p
    v_sb = kvp.tile([P, NJ_ALL, GH, HD + 1], F16)  # [s%128, j, h, d(+ones)]
    # Dedicated window-0 buffers written by the pipelined next-iteration QKV.
    kT0 = kvp.tile([P, 4, 512], F16)
    v0 = kvp.tile([P, 4, GH, HD + 1], F16)
    qT0 = kvp.tile([P, 4, 512], F16)
    xs0 = kvp.tile([P, 8, 512], F16)
    xs0_8 = kvp.tile([P, 8, 512], F8)
    xs1 = kvp.tile([P, 8, 512], F16)
    xs1_8 = kvp.tile([P, 8, 512], F8)
    ctxT3 = kvp.tile([P, 4, 512], F16)
    # First bench-loop iteration reads ctxT3 before it is ever written (the
    # pipelined qb3 outproj of a nonexistent previous iteration) — zero it.
    nc.gpsimd.memset(ctxT3[:], 0.0)

    ones_f = tmp.tile([P, NJ_ALL * GH], F32)
    nc.vector.memset(ones_f[:], 1.0)
    nc.vector.tensor_copy(
        v_sb[:, :, :, HD],
        ones_f[:].rearrange("p (j h) -> p j h", j=NJ_ALL),
    )
    nc.vector.tensor_copy(
        v0[:, :, :, HD],
        ones_f[:, 0:4 * GH].rearrange("p (j h) -> p j h", j=4),
    )
    tmpp.__exit__(None, None, None)

    xTr = xT.rearrange("(eo p) s -> p eo s", p=P)
    xTr8 = xT8.rearrange("(eo p) s -> p eo s", p=P)

    if not has_bias:
        bqk_sb = bv_sb = None

    st = dict(nc=nc, tc=tc, xTr=xTr, out=out, wqk_sb=wqk_sb, wv_sb=wv_sb,
              wo_sb=wo_sb, bqk_sb=bqk_sb, bv_sb=bv_sb, mask=mask,
              kT_sb=kT_sb, v_sb=v_sb, kT0=kT0, v0=v0, qT0=qT0, xs0=xs0,
              xs0_8=xs0_8, xs1=xs1, xs1_8=xs1_8, xTr8=xTr8, ctxT3=ctxT3, qp=qp, xp=xp, xp8=xp8, cxp=cxp, ep=ep, osb=osb, bp=bp,
              ps_s=ps_s, ps_m=ps_m, xs_by_qb={})

    # ---- prologue: window-0 QKV for the first iteration ----
    nc.sync.dma_start(xs0[:], xTr[:, :, 0:512])
    nc.sync.dma_start(xs0_8[:], xTr8[:, :, 0:512])
    nc.sync.dma_start(xs1[:], xTr[:, :, 512:1024])
    nc.sync.dma_start(xs1_8[:], xTr8[:, :, 512:1024])
    st["xs_by_qb"][0] = (xs0, xs0_8)
    st["xs_by_qb"][1] = (xs1, xs1_8)
    for chain in _qkv_strip_chains(st, 0):
        chain()

    if bench_iters:
        # 2x unrolled loop: one all-engine back-edge barrier per two
        # iterations, and the scheduler can overlap the first body's
        # out-projection tail with the second body's first window.
        unroll = 2 if bench_iters % 2 == 0 else 1
        loop_cm = tc.For_i(0, bench_iters // unroll, 1,
                           hint_engines=(mybir.EngineType.PE,
                                         mybir.EngineType.DVE,
                                         mybir.EngineType.Activation,
                                         mybir.EngineType.Pool,
                                         mybir.EngineType.SP))
        with loop_cm:
            for _ in range(unroll):
                _emit_body(st, pipelined=True)
    else:
        _emit_body(st, pipelined=False)

    for p in reversed(ctxmgr):
        p.__exit__(None, None, None)


def _publish_qb0(st):
    """Copy kT0/v0 (written during the previous iteration's last window)
    into the main kT/v banks so windows 1..3 read uniform addresses."""
    nc = st["nc"]

    def go():
        nc.vector.tensor_copy(st["kT_sb"][:, :, 0:512], st["kT0"][:])
        nc.vector.tensor_copy(st["v_sb"][:, 0:4, :, :], st["v0"][:])
    yield go


def _prefetch_x(st, qb):
    """DMA the x strip for window qb one window before its chains run.
    Strips 0/1 live in dedicated tiles (they wrap the back edge)."""
    nc = st["nc"]
    s0 = qb * 512
    if qb == 0:
        xs, xs8 = st["xs0"], st["xs0_8"]
    elif qb == 1:
        xs, xs8 = st["xs1"], st["xs1_8"]
    else:
        xs = st["xp"].tile([P, 8, 512], F16)
        xs8 = st["xp8"].tile([P, 8, 512], F8)
    st["xs_by_qb"][qb] = (xs, xs8)

    def go():
        nc.sync.dma_start(xs[:], st["xTr"][:, :, s0:s0 + 512])
        nc.sync.dma_start(xs8[:], st["xTr8"][:, :, s0:s0 + 512])
    yield go


def _emit_body(st, pipelined):
    """Software-pipelined emission: attention(qb) is the backbone; PE-only
    work — qkv(qb+1) chains and outproj(qb-1) chains — is spliced between
    individual j-iterations so the in-order PE stream always has independent
    matmuls to chew on while it waits for exp results.  In pipelined (bench
    loop) mode the qb3 outproj and the next iteration's qb0 qkv wrap around
    the loop back-edge."""
    ctx = [None, None, None, st["ctxT3"]]

    fillers = [_publish_qb0(st), _prefetch_x(st, 2), _qkv_strip_chains(st, 1)]
    ctx[0] = _attn(st, 0, fillers=_roundrobin(fillers))

    for qb in (1, 2):
        fillers = [_prefetch_x(st, (qb + 2) % 4),
                   _qkv_strip_chains(st, qb + 1),
                   _outproj_chains(st, ctx[qb - 1], qb - 1)]
        ctx[qb] = _attn(st, qb, fillers=_roundrobin(fillers))

    fillers = [_outproj_chains(st, ctx[2], 2)]
    if pipelined:
        fillers.insert(0, _qkv_strip_chains(st, 0))
        fillers.insert(0, _prefetch_x(st, 1))
    _attn(st, 3, fillers=_roundrobin(fillers))
    for chain in _outproj_chains(st, st["ctxT3"], 3):
        chain()


def _roundrobin(gens):
    gens = list(gens)
    while gens:
        g = gens.pop(0)
        try:
            yield next(g)
            gens.append(g)
        except StopIteration:
            pass


def _qkv_strip_chains(st, qb):
    """Yield one callable per accumulation chain (8 matmuls + a drain op).
    qb == 0 targets the dedicated qT0/kT0/v0 buffers (next-iteration or
    prologue window 0); other qbs target the rotating qT pool and main kT/v.
    """
    nc = st["nc"]
    wqk_sb, wv_sb = st["wqk_sb"], st["wv_sb"]
    bqk_sb = st["bqk_sb"]
    s0 = qb * 512
    state = {}

    def load_x():
        state["xs"], state["xs8"] = st["xs_by_qb"][qb]

    if qb == 0:
        qT = st["qT0"]
    else:
        qT = st["qp"].tile([P, 4, 512], F16, tag="qT", name=f"qT{qb % 2}")
    st.setdefault("qT_by_qb", {})[qb] = qT

    yield load_x

    def qk_chain(cb):
        xs8 = state["xs8"]
        pq = st["ps_m"].tile([P, 512], F32, tag="m")
        for e4 in range(4):
            nc.tensor.matmul(
                pq[:],
                wqk_sb[:, e4, :, cb * P:(cb + 1) * P],
                xs8[:, 2 * e4:2 * e4 + 2, :],
                start=(e4 == 0), stop=(e4 == 3),
                perf_mode=mybir.MatmulPerfMode.DoubleRow,
            )
        if cb < 4:
            dest = qT[:, cb, :]
        elif qb == 0:
            dest = st["kT0"][:, cb - 4, :]
        else:
            dest = st["kT_sb"][:, cb - 4, s0:s0 + 512]
        # weights were pre-scaled x64 on the host to clear fp8 subnormals
        if bqk_sb is not None:
            nc.vector.tensor_scalar(dest, pq[:], 1.0 / 64,
                                    bqk_sb[:, cb:cb + 1],
                                    op0=mybir.AluOpType.mult,
                                    op1=mybir.AluOpType.add)
        else:
            nc.vector.tensor_scalar_mul(dest, pq[:], 1.0 / 64)

    def v_chain(sv):
        xs = state["xs"]
        pv = st["ps_m"].tile([P, 512], F32, tag="m")
        for e in range(8):
            nc.tensor.matmul(
                pv[:],
                xs[:, e, sv * P:(sv + 1) * P],
                wv_sb[:, e, :],
                start=(e == 0), stop=(e == 7),
            )
        if qb == 0:
            dest = st["v0"][:, sv, :, 0:HD]
        else:
            dest = st["v_sb"][:, s0 // P + sv, :, 0:HD]
        nc.vector.tensor_copy(dest, pv[:].rearrange("p (h d) -> p h d", h=GH))

    # k and v chains first: the next window's attention needs kT/v before qT
    for cb in (4, 5, 6, 7):
        yield (lambda cb=cb: qk_chain(cb))
    for sv in range(4):
        yield (lambda sv=sv: v_chain(sv))
    for cb in (0, 1, 2, 3):
        yield (lambda cb=cb: qk_chain(cb))


def _emit_pv(st, pv2, c, use0, j, w0, ex, nj):
    nc = st["nc"]
    for hp in range(2):
        src = (st["v0"][:, j, 2 * c + hp, :] if use0
               else st["v_sb"][:, j, 2 * c + hp, :])
        nc.tensor.matmul(
            pv2[hp][0:HD + 1, w0:512],
            src,
            ex[:, hp, w0:512],
            start=(j == 0), stop=(j == nj - 1),
        )


def _outproj_chains(st, ctxT, qb):
    nc = st["nc"]
    q0 = qb * 512
    for sb_i in range(4):
        for ob in range(2):
            def chain(sb_i=sb_i, ob=ob):
                po = st["ps_m"].tile([P, 512], F32, tag="m")
                for cc in range(4):
                    nc.tensor.matmul(
                        po[:],
                        ctxT[:, cc, sb_i * P:(sb_i + 1) * P],
                        st["wo_sb"][:, cc, ob * 512:(ob + 1) * 512],
                        start=(cc == 0), stop=(cc == 3),
                    )
                ot = st["osb"].tile([P, 512], F16)
                nc.vector.tensor_copy(ot[:], po[:])
                nc.sync.dma_start(
                    st["out"][q0 + sb_i * P:q0 + (sb_i + 1) * P,
                              ob * 512:(ob + 1) * 512],
                    ot[:],
                )
            yield chain


def _attn(st, qb, fillers=None):
    """Attention for q-window qb. Heads 2c (SBUF partitions 0-63) and 2c+1
    (64-127) are processed together. Returns the ctxT tile."""
    nc = st["nc"]
    mask = st["mask"]
    bv_sb = st["bv_sb"]
    qT = st["qT_by_qb"][qb]
    if qb == 3:
        ctxT = st["ctxT3"]
    else:
        ctxT = st["cxp"].tile([P, 4, 512], F16)
    nj = 4 * (qb + 1)
    fillers = iter(fillers) if fillers is not None else iter(())
    done = False
    n_iters = 4 * nj
    acc = 0.0
    per_iter = 22.0 / n_iters   # ~22 filler chains spread over the window

    def emit_fillers(force_all=False):
        nonlocal acc, done
        if done:
            return
        acc += per_iter
        while (acc >= 1.0 or force_all) and not done:
            acc -= 1.0
            try:
                next(fillers)()
            except StopIteration:
                done = True

    for c in range(4):
        pv2 = [st["ps_m"].tile([P, 512], F32, tag="m", name=f"pv{hp}")
               for hp in range(2)]
        pend = None   # software-pipeline: PV trails scores by one j
        for j in range(nj):
            t = j - 4 * qb
            # Diagonal blocks only need q-columns >= t*128 (causality).
            w0 = 0 if t < 0 else t * P
            sp = st["ps_s"].tile([P, 2, 512], F32)
            for hp in range(2):
                p0 = 64 * hp
                src = (st["kT0"][p0:p0 + 64, c, j * P:(j + 1) * P] if qb == 0
                       else st["kT_sb"][p0:p0 + 64, c, j * P:(j + 1) * P])
                nc.tensor.matmul(
                    sp[:, hp, w0:512],
                    src,
                    qT[p0:p0 + 64, c, w0:512],
                    start=True, stop=True,
                )
            ex = st["ep"].tile([P, 2, 512], F16)
            nc.scalar.activation(ex[:, :, w0:512], sp[:, :, w0:512], AF.Exp)
            if t >= 0:
                # mask multiply over the diagonal band [t*128, (t+1)*128)
                m0 = t * P
                nc.vector.tensor_mul(
                    ex[:, :, m0:m0 + P],
                    ex[:, :, m0:m0 + P],
                    mask[:, None, :].to_broadcast((P, 2, P)),
                )
            if pend is not None:
                _emit_pv(st, pv2, c, qb == 0, *pend, nj)
            pend = (j, w0, ex)
            emit_fillers()
        if pend is not None:
            _emit_pv(st, pv2, c, qb == 0, *pend, nj)
        # normalize: ctxT = pv[0:64] / pv[64] (+ v bias)
        for hp in range(2):
            p0 = 64 * hp
            pv_ps = pv2[hp]
            bc = st["bp"].tile([64, 512], F32)
            nc.vector.reciprocal(bc[0:1, :], pv_ps[HD:HD + 1, :])
            nc.gpsimd.partition_broadcast(bc[:], bc[0:1, :])
            nc.vector.tensor_mul(ctxT[p0:p0 + 64, c, :], pv_ps[0:HD, :], bc[:])
            if bv_sb is not None:
                nc.vector.tensor_scalar_add(
                    ctxT[p0:p0 + 64, c, :],
                    ctxT[p0:p0 + 64, c, :],
                    bv_sb[p0:p0 + 64, c:c + 1],
                )
    emit_fillers(force_all=True)
    return ctxT


def _build_program(bench_iters=0, has_bias=True):
    nc = bacc.Bacc("TRN2", target_bir_lowering=False, debug=False,
                   num_devices=NCORES)
    xT = nc.dram_tensor("xT", [E, S], F16, kind="ExternalInput").ap()
    xT8 = nc.dram_tensor("xT8", [E, S], F8, kind="ExternalInput").ap()
    wqkT = nc.dram_tensor("wqkT", [E, 2 * GC], F8, kind="ExternalInput").ap()
    wvT = nc.dram_tensor("wvT", [E, GC], F16, kind="ExternalInput").ap()
    woT = nc.dram_tensor("woT", [GC, E], F16, kind="ExternalInput").ap()
    bqk = nc.dram_tensor("bqk", [8, P], F32, kind="ExternalInput").ap()
    bv = nc.dram_tensor("bv", [4, P], F32, kind="ExternalInput").ap()
    out = nc.dram_tensor("o", [S, E], F16, kind="ExternalOutput").ap()
    with tile.TileContext(nc) as tc:
        _emit(tc, nc, xT, xT8, wqkT, wvT, woT, bqk, bv, out,
              bench_iters=bench_iters, has_bias=has_bias)
    nc.compile()
    return nc


def _get_program(has_bias=True):
    if has_bias not in _program:
        _program[has_bias] = _build_program(has_bias=has_bias)
    return _program[has_bias]


def _make_in_maps(x, in_proj_w, in_proj_b, out_proj_w):
    scale = np.float32(1.0 / np.sqrt(HD))
    in_maps = []
    for c in range(NCORES):
        b, g = divmod(c, 2)
        lo, hi = g * GC, (g + 1) * GC
        wq = in_proj_w[lo:hi, :]
        wk = in_proj_w[E + lo:E + hi, :]
        wv = in_proj_w[2 * E + lo:2 * E + hi, :]
        wqkT = np.concatenate([wq.T * scale, wk.T], axis=1)
        bqk = np.concatenate([in_proj_b[lo:hi] * scale,
                              in_proj_b[E + lo:E + hi]]).reshape(8, P)
        bvv = in_proj_b[2 * E + lo:2 * E + hi].reshape(4, P)
        in_maps.append({
            "xT": np.ascontiguousarray(x[b].T).astype(NP16),
            "xT8": np.ascontiguousarray(x[b].T).astype(NP8),
            "wqkT": np.ascontiguousarray(wqkT * 64.0).astype(NP8),
            "wvT": np.ascontiguousarray(wv.T).astype(NP16),
            "woT": np.ascontiguousarray(out_proj_w[:, lo:hi].T).astype(NP16),
            "bqk": np.ascontiguousarray(bqk, dtype=np.float32),
            "bv": np.ascontiguousarray(bvv, dtype=np.float32),
        })
    return in_maps


def _combine(results, out_proj_b):
    out = np.empty((B, S, E), dtype=np.float32)
    for b in range(B):
        out[b] = (results[2 * b]["o"].astype(np.float32)
                  + results[2 * b + 1]["o"].astype(np.float32))
    out += np.asarray(out_proj_b, dtype=np.float32)[None, None, :]
    return out


def kernel(x, in_proj_w, in_proj_b, out_proj_w, out_proj_b, _trace=False):
    x = np.asarray(x, dtype=np.float32)
    in_proj_w = np.asarray(in_proj_w, dtype=np.float32)
    in_proj_b = np.asarray(in_proj_b, dtype=np.float32)
    out_proj_w = np.asarray(out_proj_w, dtype=np.float32)
    out_proj_b = np.asarray(out_proj_b, dtype=np.float32)
    assert x.shape == (B, S, E), x.shape

    has_bias = bool(np.any(in_proj_b))
    nc = _get_program(has_bias=has_bias)
    in_maps = _make_in_maps(x, in_proj_w, in_proj_b, out_proj_w)
    res = run_bass_kernel_spmd(nc, in_maps, core_ids=list(range(NCORES)),
                               trace=_trace)
    out = _combine(res.results, out_proj_b)
    if _trace:
        return out, res
    return out


# revision 5
# speedup vs baseline: 1.0119x; 1.0119x over previous
""" Causal multi-head attention (B=4, S=2048, E=1024, H=16) on 8 trn2 NeuronCores.

Sharding: core c = (batch b = c//2, head-group g = c%2). Each core computes
attention for one batch element and 8 of the 16 heads, plus the partial
out-projection restricted to its heads' columns. Host sums the two partial
outputs per batch element and adds the out-projection bias.

v2 over the baseline:
  - fp16 operands everywhere (PSUM accum stays fp32): halves SBUF/DMA/DVE
    cost and lifts the fp32r moving>=256 constraint so diagonal blocks trim
    to exactly the causal band.
  - cross-iteration software pipelining: the next iteration's first q-window
    QKV runs as PE filler during the last attention window, writing
    dedicated qT0/kT0/v0 buffers (no WAR hazard with the current iteration);
    cheap DVE copies publish kT0/v0 into the main kT/v banks during window 0.
    The qb3 out-projection likewise slides into the next iteration's window 0.
    This keeps the PE instruction stream gapless across the loop back-edge
    (PE pstate drops to 1.2 GHz after any idle and needs 3us busy to
    recover 2.4 GHz, so gaps cost ~2x on every matmul that follows).

Dataflow (all "transposed" so no on-device transposes):
  qT, kT  [ch, s]   from  W_chunk @ x.T        (lhsT = W.T tiles, rhs = x.T)
  v       [s, ch]   from  x @ Wv.T             (lhsT = x.T tiles, rhs = Wv.T)
  scoresT [k, q]    from  lhsT = kT, rhs = qT  (per head, K = 64)
  ctxT    [d, q]    from  lhsT = v (+ones col), rhs = exp(scoresT)
  outP    [s, o]    from  lhsT = ctxT, rhs = Wo.T
Softmax without max subtraction (scores bounded ~|2|); normalizer from a
ones-column in v; causal masking via one multiplicative [128,128] triangular
mask tile applied to the exp'd diagonal blocks.
"""

import sys

sys.path.insert(0, "/opt/trn_rl_repo")

import numpy as np

import concourse.bass as bass  # noqa: F401  (registers engine classes)
import concourse.mybir as mybir
import concourse.tile as tile
from concourse import bacc
from concourse.bass_utils import run_bass_kernel_spmd

F32 = mybir.dt.float32
F8 = mybir.dt.float8e4
NP8 = None  # set below
F16 = mybir.dt.float16
NP16 = np.float16
NP8 = mybir.dt.np(mybir.dt.float8e4)
AF = mybir.ActivationFunctionType

B, S, E = 4, 2048, 1024
H, HD = 16, 64
GH = 8                 # heads handled per core
GC = GH * HD           # 512 channels per head-group
P = 128
NCORES = 8
NJ_ALL = S // P        # 16 k-blocks of 128
QB = S // 512          # 4 q-windows of 512

_program = {}


def _emit(tc, nc, xT, xT8, wqkT, wvT, woT, bqk, bv, out, bench_iters=0,
          has_bias=True):
    ctxmgr = []

    def pool(**kw):
        p = tc.tile_pool(**kw)
        ctxmgr.append(p)
        return p.__enter__()

    const = pool(name="const", bufs=1)
    kvp = pool(name="kv", bufs=1)
    xp = pool(name="xs", bufs=2)
    xp8 = pool(name="xs8", bufs=2)
    qp = pool(name="qt", bufs=2)
    cxp = pool(name="ctx", bufs=2)
    ep = pool(name="expt", bufs=6)
    osb = pool(name="osb", bufs=2)
    bp = pool(name="bcast", bufs=2)
    ps_s = pool(name="ps_s", bufs=2, space="PSUM")
    ps_m = pool(name="ps_m", bufs=4, space="PSUM")

    # ---- constants ----
    # DMA order matters at startup: the first qkT matmuls need wqk + the
    # first x strip; wo is only needed later, so it is emitted last.
    bqk_sb = const.tile([P, 8], F32)
    nc.sync.dma_start(bqk_sb[:], bqk.rearrange("c p -> p c"))
    bv_sb = const.tile([P, 4], F32)
    nc.sync.dma_start(bv_sb[:], bv.rearrange("c p -> p c"))
    wqk_sb = const.tile([P, 4, 2, 2 * GC], F8)    # [p, e4, i2, ch]
    wqk_r = wqkT.rearrange("(eo p) c -> p eo c", p=P)
    for e in range(8):
        eng = nc.sync if e % 2 == 0 else nc.gpsimd
        eng.dma_start(wqk_sb[:, e // 2, e % 2, :], wqk_r[:, e, :])
    wv_sb = const.tile([P, 8, GC], F16)
    wv_r = wvT.rearrange("(eo p) c -> p eo c", p=P)
    for e in range(8):
        eng = nc.gpsimd if e % 2 == 0 else nc.sync
        eng.dma_start(wv_sb[:, e, :], wv_r[:, e, :])
    wo_sb = const.tile([P, 4, E], F16)
    wo_r = woT.rearrange("(co p) o -> p co o", p=P)
    for co in range(4):
        nc.gpsimd.dma_start(wo_sb[:, co, :], wo_r[:, co, :])

    # Causal mask for diagonal 128-blocks: mask[p, u] = 1 if u >= p else 0
    # (p = key within block, u = query within block).
    tmpp = tc.tile_pool(name="tmpf", bufs=1)
    tmp = tmpp.__enter__()
    mask_f = tmp.tile([P, P], F32)
    nc.gpsimd.memset(mask_f[:], 1.0)
    nc.gpsimd.affine_select(
        out=mask_f[:],
        in_=mask_f[:],
        compare_op=mybir.AluOpType.is_ge,
        fill=0.0,
        base=0,
        pattern=[[1, P]],        # + u
        channel_multiplier=-1,   # - p   => keep where u - p >= 0
    )
    mask = const.tile([P, P], F16)
    nc.vector.tensor_copy(mask[:], mask_f[:])

    # ---- persistent tensors ----
    # kT holds k-blocks 4..15 (cols 512:2048); blocks 0..3 live in kT0/the
    # per-iteration copy, see below.
    kT_sb = kvp.tile([P, 4, S], F16)             # [p, c, s]; ch = c*128+p
    v_sb = kvp.tile([P, NJ_ALL, GH, HD + 1], F16)  # [s%128, j, h, d(+ones)]
    # Dedicated window-0 buffers written by the pipelined next-iteration QKV.
    kT0 = kvp.tile([P, 4, 512], F16)
    v0 = kvp.tile([P, 4, GH, HD + 1], F16)
    qT0 = kvp.tile([P, 4, 512], F16)
    xs0 = kvp.tile([P, 8, 512], F16)
    xs0_8 = kvp.tile([P, 8, 512], F8)
    xs1 = kvp.tile([P, 8, 512], F16)
    xs1_8 = kvp.tile([P, 8, 512], F8)
    ctxT3 = kvp.tile([P, 4, 512], F16)
    # First bench-loop iteration reads ctxT3 before it is ever written (the
    # pipelined qb3 outproj of a nonexistent previous iteration) — zero it.
    nc.gpsimd.memset(ctxT3[:], 0.0)

    ones_f = tmp.tile([P, NJ_ALL * GH], F32)
    nc.vector.memset(ones_f[:], 1.0)
    nc.vector.tensor_copy(
        v_sb[:, :, :, HD],
        ones_f[:].rearrange("p (j h) -> p j h", j=NJ_ALL),
    )
    nc.vector.tensor_copy(
        v0[:, :, :, HD],
        ones_f[:, 0:4 * GH].rearrange("p (j h) -> p j h", j=4),
    )
    tmpp.__exit__(None, None, None)

    xTr = xT.rearrange("(eo p) s -> p eo s", p=P)
    xTr8 = xT8.rearrange("(eo p) s -> p eo s", p=P)

    if not has_bias:
        bqk_sb = bv_sb = None

    st = dict(nc=nc, tc=tc, xTr=xTr, out=out, wqk_sb=wqk_sb, wv_sb=wv_sb,
              wo_sb=wo_sb, bqk_sb=bqk_sb, bv_sb=bv_sb, mask=mask,
              kT_sb=kT_sb, v_sb=v_sb, kT0=kT0, v0=v0, qT0=qT0, xs0=xs0,
              xs0_8=xs0_8, xs1=xs1, xs1_8=xs1_8, xTr8=xTr8, ctxT3=ctxT3, qp=qp, xp=xp, xp8=xp8, cxp=cxp, ep=ep, osb=osb, bp=bp,
              ps_s=ps_s, ps_m=ps_m, xs_by_qb={})

    # ---- prologue: window-0 QKV for the first iteration ----
    nc.sync.dma_start(xs0[:], xTr[:, :, 0:512])
    nc.sync.dma_start(xs0_8[:], xTr8[:, :, 0:512])
    nc.sync.dma_start(xs1[:], xTr[:, :, 512:1024])
    nc.sync.dma_start(xs1_8[:], xTr8[:, :, 512:1024])
    st["xs_by_qb"][0] = (xs0, xs0_8)
    st["xs_by_qb"][1] = (xs1, xs1_8)
    for chain in _qkv_strip_chains(st, 0):
        chain()

    if bench_iters:
        # 2x unrolled loop: one all-engine back-edge barrier per two
        # iterations. The first body defers its qb3 out-projection into the
        # second body's window-0 fillers (no barrier between the two), so
        # only the second body's out-projection tail sits before the barrier.
        if bench_iters % 2 == 0:
            loop_cm = tc.For_i(0, bench_iters // 2, 1,
                               hint_engines=(mybir.EngineType.PE,
                                             mybir.EngineType.DVE,
                                             mybir.EngineType.Activation,
                                             mybir.EngineType.Pool,
                                             mybir.EngineType.SP))
            with loop_cm:
                _emit_body(st, pipelined=True, defer_out3=True)
                _emit_body(st, pipelined=True, take_deferred=True)
        else:
            loop_cm = tc.For_i(0, bench_iters, 1,
                               hint_engines=(mybir.EngineType.PE,
                                             mybir.EngineType.DVE,
                                             mybir.EngineType.Activation,
                                             mybir.EngineType.Pool,
                                             mybir.EngineType.SP))
            with loop_cm:
                _emit_body(st, pipelined=True)
    else:
        _emit_body(st, pipelined=False)

    for p in reversed(ctxmgr):
        p.__exit__(None, None, None)


def _publish_qb0(st):
    """Copy kT0/v0 (written during the previous iteration's last window)
    into the main kT/v banks so windows 1..3 read uniform addresses."""
    nc = st["nc"]

    def go():
        nc.vector.tensor_copy(st["kT_sb"][:, :, 0:512], st["kT0"][:])
        nc.vector.tensor_copy(st["v_sb"][:, 0:4, :, :], st["v0"][:])
    yield go


def _prefetch_x(st, qb):
    """DMA the x strip for window qb one window before its chains run.
    Strips 0/1 live in dedicated tiles (they wrap the back edge)."""
    nc = st["nc"]
    s0 = qb * 512
    if qb == 0:
        xs, xs8 = st["xs0"], st["xs0_8"]
    elif qb == 1:
        xs, xs8 = st["xs1"], st["xs1_8"]
    else:
        xs = st["xp"].tile([P, 8, 512], F16)
        xs8 = st["xp8"].tile([P, 8, 512], F8)
    st["xs_by_qb"][qb] = (xs, xs8)

    def go():
        nc.sync.dma_start(xs[:], st["xTr"][:, :, s0:s0 + 512])
        nc.sync.dma_start(xs8[:], st["xTr8"][:, :, s0:s0 + 512])
    yield go


def _emit_body(st, pipelined, defer_out3=False, take_deferred=False):
    """Software-pipelined emission: attention(qb) is the backbone; PE-only
    work — qkv(qb+1) chains and outproj(qb-1) chains — is spliced between
    individual j-iterations so the in-order PE stream always has independent
    matmuls to chew on while it waits for exp results.  In pipelined (bench
    loop) mode the qb3 outproj and the next iteration's qb0 qkv wrap around
    the loop back-edge."""
    ctx = [None, None, None, st["ctxT3"]]

    fillers = [_publish_qb0(st), _prefetch_x(st, 2), _qkv_strip_chains(st, 1)]
    if take_deferred:
        # previous body in the unrolled pair deferred its qb3 outproj here;
        # no barrier sits between the two bodies, so this overlaps window 0.
        fillers.append(_outproj_chains(st, st["ctxT3"], 3))
    ctx[0] = _attn(st, 0, fillers=_roundrobin(fillers))

    for qb in (1, 2):
        fillers = [_prefetch_x(st, (qb + 2) % 4),
                   _qkv_strip_chains(st, qb + 1),
                   _outproj_chains(st, ctx[qb - 1], qb - 1)]
        ctx[qb] = _attn(st, qb, fillers=_roundrobin(fillers))

    fillers = [_outproj_chains(st, ctx[2], 2)]
    if pipelined:
        fillers.insert(0, _qkv_strip_chains(st, 0))
        fillers.insert(0, _prefetch_x(st, 1))
    _attn(st, 3, fillers=_roundrobin(fillers))
    if not defer_out3:
        for chain in _outproj_chains(st, st["ctxT3"], 3):
            chain()


def _roundrobin(gens):
    gens = list(gens)
    while gens:
        g = gens.pop(0)
        try:
            yield next(g)
            gens.append(g)
        except StopIteration:
            pass


def _qkv_strip_chains(st, qb):
    """Yield one callable per accumulation chain (8 matmuls + a drain op).
    qb == 0 targets the dedicated qT0/kT0/v0 buffers (next-iteration or
    prologue window 0); other qbs target the rotating qT pool and main kT/v.
    """
    nc = st["nc"]
    wqk_sb, wv_sb = st["wqk_sb"], st["wv_sb"]
    bqk_sb = st["bqk_sb"]
    s0 = qb * 512
    state = {}

    def load_x():
        state["xs"], state["xs8"] = st["xs_by_qb"][qb]

    if qb == 0:
        qT = st["qT0"]
    else:
        qT = st["qp"].tile([P, 4, 512], F16, tag="qT", name=f"qT{qb % 2}")
    st.setdefault("qT_by_qb", {})[qb] = qT

    yield load_x

    def qk_chain(cb):
        xs8 = state["xs8"]
        pq = st["ps_m"].tile([P, 512], F32, tag="m")
        for e4 in range(4):
            nc.tensor.matmul(
                pq[:],
                wqk_sb[:, e4, :, cb * P:(cb + 1) * P],
                xs8[:, 2 * e4:2 * e4 + 2, :],
                start=(e4 == 0), stop=(e4 == 3),
                perf_mode=mybir.MatmulPerfMode.DoubleRow,
            )
        if cb < 4:
            dest = qT[:, cb, :]
        elif qb == 0:
            dest = st["kT0"][:, cb - 4, :]
        else:
            dest = st["kT_sb"][:, cb - 4, s0:s0 + 512]
        # weights were pre-scaled x64 on the host to clear fp8 subnormals
        if bqk_sb is not None:
            nc.vector.tensor_scalar(dest, pq[:], 1.0 / 64,
                                    bqk_sb[:, cb:cb + 1],
                                    op0=mybir.AluOpType.mult,
                                    op1=mybir.AluOpType.add)
        else:
            nc.vector.tensor_scalar_mul(dest, pq[:], 1.0 / 64)

    def v_chain(sv):
        xs = state["xs"]
        pv = st["ps_m"].tile([P, 512], F32, tag="m")
        for e in range(8):
            nc.tensor.matmul(
                pv[:],
                xs[:, e, sv * P:(sv + 1) * P],
                wv_sb[:, e, :],
                start=(e == 0), stop=(e == 7),
            )
        if qb == 0:
            dest = st["v0"][:, sv, :, 0:HD]
        else:
            dest = st["v_sb"][:, s0 // P + sv, :, 0:HD]
        nc.vector.tensor_copy(dest, pv[:].rearrange("p (h d) -> p h d", h=GH))

    # k and v chains first: the next window's attention needs kT/v before qT
    for cb in (4, 5, 6, 7):
        yield (lambda cb=cb: qk_chain(cb))
    for sv in range(4):
        yield (lambda sv=sv: v_chain(sv))
    for cb in (0, 1, 2, 3):
        yield (lambda cb=cb: qk_chain(cb))


def _emit_pv(st, pv2, c, use0, j, w0, ex, nj):
    nc = st["nc"]
    for hp in range(2):
        src = (st["v0"][:, j, 2 * c + hp, :] if use0
               else st["v_sb"][:, j, 2 * c + hp, :])
        nc.tensor.matmul(
            pv2[hp][0:HD + 1, w0:512],
            src,
            ex[:, hp, w0:512],
            start=(j == 0), stop=(j == nj - 1),
        )


def _outproj_chains(st, ctxT, qb):
    nc = st["nc"]
    q0 = qb * 512
    for sb_i in range(4):
        for ob in range(2):
            def chain(sb_i=sb_i, ob=ob):
                po = st["ps_m"].tile([P, 512], F32, tag="m")
                for cc in range(4):
                    nc.tensor.matmul(
                        po[:],
                        ctxT[:, cc, sb_i * P:(sb_i + 1) * P],
                        st["wo_sb"][:, cc, ob * 512:(ob + 1) * 512],
                        start=(cc == 0), stop=(cc == 3),
                    )
                ot = st["osb"].tile([P, 512], F16)
                nc.vector.tensor_copy(ot[:], po[:])
                nc.sync.dma_start(
                    st["out"][q0 + sb_i * P:q0 + (sb_i + 1) * P,
                              ob * 512:(ob + 1) * 512],
                    ot[:],
                )
            yield chain


def _attn(st, qb, fillers=None):
    """Attention for q-window qb. Heads 2c (SBUF partitions 0-63) and 2c+1
    (64-127) are processed together. Returns the ctxT tile."""
    nc = st["nc"]
    mask = st["mask"]
    bv_sb = st["bv_sb"]
    qT = st["qT_by_qb"][qb]
    if qb == 3:
        ctxT = st["ctxT3"]
    else:
        ctxT = st["cxp"].tile([P, 4, 512], F16)
    nj = 4 * (qb + 1)
    fillers = iter(fillers) if fillers is not None else iter(())
    done = False
    n_iters = 4 * nj
    acc = 0.0
    per_iter = 22.0 / n_iters   # ~22 filler chains spread over the window

    def emit_fillers(force_all=False):
        nonlocal acc, done
        if done:
            return
        acc += per_iter
        while (acc >= 1.0 or force_all) and not done:
            acc -= 1.0
            try:
                next(fillers)()
            except StopIteration:
                done = True

    for c in range(4):
        pv2 = [st["ps_m"].tile([P, 512], F32, tag="m", name=f"pv{hp}")
               for hp in range(2)]
        pend = None   # software-pipeline: PV trails scores by one j
        for j in range(nj):
            t = j - 4 * qb
            # Diagonal blocks only need q-columns >= t*128 (causality).
            w0 = 0 if t < 0 else t * P
            sp = st["ps_s"].tile([P, 2, 512], F32)
            for hp in range(2):
                p0 = 64 * hp
                src = (st["kT0"][p0:p0 + 64, c, j * P:(j + 1) * P] if qb == 0
                       else st["kT_sb"][p0:p0 + 64, c, j * P:(j + 1) * P])
                nc.tensor.matmul(
                    sp[:, hp, w0:512],
                    src,
                    qT[p0:p0 + 64, c, w0:512],
                    start=True, stop=True,
                )
            ex = st["ep"].tile([P, 2, 512], F16)
            nc.scalar.activation(ex[:, :, w0:512], sp[:, :, w0:512], AF.Exp)
            if t >= 0:
                # mask multiply over the diagonal band [t*128, (t+1)*128)
                m0 = t * P
                nc.vector.tensor_mul(
                    ex[:, :, m0:m0 + P],
                    ex[:, :, m0:m0 + P],
                    mask[:, None, :].to_broadcast((P, 2, P)),
                )
            if pend is not None:
                _emit_pv(st, pv2, c, qb == 0, *pend, nj)
            pend = (j, w0, ex)
            emit_fillers()
        if pend is not None:
            _emit_pv(st, pv2, c, qb == 0, *pend, nj)
        # normalize: ctxT = pv[0:64] / pv[64] (+ v bias)
        for hp in range(2):
            p0 = 64 * hp
            pv_ps = pv2[hp]
            bc = st["bp"].tile([64, 512], F32)
            nc.vector.reciprocal(bc[0:1, :], pv_ps[HD:HD + 1, :])
            nc.gpsimd.partition_broadcast(bc[:], bc[0:1, :])
            nc.vector.tensor_mul(ctxT[p0:p0 + 64, c, :], pv_ps[0:HD, :], bc[:])
            if bv_sb is not None:
                nc.vector.tensor_scalar_add(
                    ctxT[p0:p0 + 64, c, :],
                    ctxT[p0:p0 + 64, c, :],
                    bv_sb[p0:p0 + 64, c:c + 1],
                )
    emit_fillers(force_all=True)
    return ctxT


def _build_program(bench_iters=0, has_bias=True):
    nc = bacc.Bacc("TRN2", target_bir_lowering=False, debug=False,
                   num_devices=NCORES)
    xT = nc.dram_tensor("xT", [E, S], F16, kind="ExternalInput").ap()
    xT8 = nc.dram_tensor("xT8", [E, S], F8, kind="ExternalInput").ap()
    wqkT = nc.dram_tensor("wqkT", [E, 2 * GC], F8, kind="ExternalInput").ap()
    wvT = nc.dram_tensor("wvT", [E, GC], F16, kind="ExternalInput").ap()
    woT = nc.dram_tensor("woT", [GC, E], F16, kind="ExternalInput").ap()
    bqk = nc.dram_tensor("bqk", [8, P], F32, kind="ExternalInput").ap()
    bv = nc.dram_tensor("bv", [4, P], F32, kind="ExternalInput").ap()
    out = nc.dram_tensor("o", [S, E], F16, kind="ExternalOutput").ap()
    with tile.TileContext(nc) as tc:
        _emit(tc, nc, xT, xT8, wqkT, wvT, woT, bqk, bv, out,
              bench_iters=bench_iters, has_bias=has_bias)
    nc.compile()
    return nc


def _get_program(has_bias=True):
    if has_bias not in _program:
        _program[has_bias] = _build_program(has_bias=has_bias)
    return _program[has_bias]


def _make_in_maps(x, in_proj_w, in_proj_b, out_proj_w):
    scale = np.float32(1.0 / np.sqrt(HD))
    in_maps = []
    for c in range(NCORES):
        b, g = divmod(c, 2)
        lo, hi = g * GC, (g + 1) * GC
        wq = in_proj_w[lo:hi, :]
        wk = in_proj_w[E + lo:E + hi, :]
        wv = in_proj_w[2 * E + lo:2 * E + hi, :]
        wqkT = np.concatenate([wq.T * scale, wk.T], axis=1)
        bqk = np.concatenate([in_proj_b[lo:hi] * scale,
                              in_proj_b[E + lo:E + hi]]).reshape(8, P)
        bvv = in_proj_b[2 * E + lo:2 * E + hi].reshape(4, P)
        in_maps.append({
            "xT": np.ascontiguousarray(x[b].T).astype(NP16),
            "xT8": np.ascontiguousarray(x[b].T).astype(NP8),
            "wqkT": np.ascontiguousarray(wqkT * 64.0).astype(NP8),
            "wvT": np.ascontiguousarray(wv.T).astype(NP16),
            "woT": np.ascontiguousarray(out_proj_w[:, lo:hi].T).astype(NP16),
            "bqk": np.ascontiguousarray(bqk, dtype=np.float32),
            "bv": np.ascontiguousarray(bvv, dtype=np.float32),
        })
    return in_maps


def _combine(results, out_proj_b):
    out = np.empty((B, S, E), dtype=np.float32)
    for b in range(B):
        out[b] = (results[2 * b]["o"].astype(np.float32)
                  + results[2 * b + 1]["o"].astype(np.float32))
    out += np.asarray(out_proj_b, dtype=np.float32)[None, None, :]
    return out


def kernel(x, in_proj_w, in_proj_b, out_proj_w, out_proj_b, _trace=False):
    x = np.asarray(x, dtype=np.float32)
    in_proj_w = np.asarray(in_proj_w, dtype=np.float32)
    in_proj_b = np.asarray(in_proj_b, dtype=np.float32)
    out_proj_w = np.asarray(out_proj_w, dtype=np.float32)
    out_proj_b = np.asarray(out_proj_b, dtype=np.float32)
    assert x.shape == (B, S, E), x.shape

    has_bias = bool(np.any(in_proj_b))
    nc = _get_program(has_bias=has_bias)
    in_maps = _make_in_maps(x, in_proj_w, in_proj_b, out_proj_w)
    res = run_bass_kernel_spmd(nc, in_maps, core_ids=list(range(NCORES)),
                               trace=_trace)
    out = _combine(res.results, out_proj_b)
    if _trace:
        return out, res
    return out


# revision 6
# speedup vs baseline: 1.0138x; 1.0019x over previous
""" Causal multi-head attention (B=4, S=2048, E=1024, H=16) on 8 trn2 NeuronCores.

Sharding: core c = (batch b = c//2, head-group g = c%2). Each core computes
attention for one batch element and 8 of the 16 heads, plus the partial
out-projection restricted to its heads' columns. Host sums the two partial
outputs per batch element and adds the out-projection bias.

v2 over the baseline:
  - fp16 operands everywhere (PSUM accum stays fp32): halves SBUF/DMA/DVE
    cost and lifts the fp32r moving>=256 constraint so diagonal blocks trim
    to exactly the causal band.
  - cross-iteration software pipelining: the next iteration's first q-window
    QKV runs as PE filler during the last attention window, writing
    dedicated qT0/kT0/v0 buffers (no WAR hazard with the current iteration);
    cheap DVE copies publish kT0/v0 into the main kT/v banks during window 0.
    The qb3 out-projection likewise slides into the next iteration's window 0.
    This keeps the PE instruction stream gapless across the loop back-edge
    (PE pstate drops to 1.2 GHz after any idle and needs 3us busy to
    recover 2.4 GHz, so gaps cost ~2x on every matmul that follows).

Dataflow (all "transposed" so no on-device transposes):
  qT, kT  [ch, s]   from  W_chunk @ x.T        (lhsT = W.T tiles, rhs = x.T)
  v       [s, ch]   from  x @ Wv.T             (lhsT = x.T tiles, rhs = Wv.T)
  scoresT [k, q]    from  lhsT = kT, rhs = qT  (per head, K = 64)
  ctxT    [d, q]    from  lhsT = v (+ones col), rhs = exp(scoresT)
  outP    [s, o]    from  lhsT = ctxT, rhs = Wo.T
Softmax without max subtraction (scores bounded ~|2|); normalizer from a
ones-column in v; causal masking via one multiplicative [128,128] triangular
mask tile applied to the exp'd diagonal blocks.
"""

import sys

sys.path.insert(0, "/opt/trn_rl_repo")

import numpy as np

import concourse.bass as bass  # noqa: F401  (registers engine classes)
import concourse.mybir as mybir
import concourse.tile as tile
from concourse import bacc
from concourse.bass_utils import run_bass_kernel_spmd

F32 = mybir.dt.float32
F8 = mybir.dt.float8e4
NP8 = None  # set below
F16 = mybir.dt.float16
NP16 = np.float16
NP8 = mybir.dt.np(mybir.dt.float8e4)
AF = mybir.ActivationFunctionType

B, S, E = 4, 2048, 1024
H, HD = 16, 64
GH = 8                 # heads handled per core
GC = GH * HD           # 512 channels per head-group
P = 128
NCORES = 8
NJ_ALL = S // P        # 16 k-blocks of 128
QB = S // 512          # 4 q-windows of 512

_program = {}


def _emit(tc, nc, xT, xT8, wqkT, wvT, woT, bqk, bv, out, bench_iters=0,
          has_bias=True):
    ctxmgr = []

    def pool(**kw):
        p = tc.tile_pool(**kw)
        ctxmgr.append(p)
        return p.__enter__()

    const = pool(name="const", bufs=1)
    kvp = pool(name="kv", bufs=1)
    xp = pool(name="xs", bufs=2)
    xp8 = pool(name="xs8", bufs=2)
    qp = pool(name="qt", bufs=2)
    cxp = pool(name="ctx", bufs=2)
    ep = pool(name="expt", bufs=6)
    osb = pool(name="osb", bufs=2)
    bp = pool(name="bcast", bufs=2)
    ps_s = pool(name="ps_s", bufs=2, space="PSUM")
    ps_m = pool(name="ps_m", bufs=4, space="PSUM")

    # ---- constants ----
    # DMA order matters at startup: the first qkT matmuls need wqk + the
    # first x strip; wo is only needed later, so it is emitted last.
    bqk_sb = const.tile([P, 8], F32)
    nc.sync.dma_start(bqk_sb[:], bqk.rearrange("c p -> p c"))
    bv_sb = const.tile([P, 4], F32)
    nc.sync.dma_start(bv_sb[:], bv.rearrange("c p -> p c"))
    wqk_sb = const.tile([P, 4, 2, 2 * GC], F8)    # [p, e4, i2, ch]
    wqk_r = wqkT.rearrange("(eo p) c -> p eo c", p=P)
    for e in range(8):
        eng = nc.sync if e % 2 == 0 else nc.gpsimd
        eng.dma_start(wqk_sb[:, e // 2, e % 2, :], wqk_r[:, e, :])
    wv_sb = const.tile([P, 8, GC], F16)
    wv_r = wvT.rearrange("(eo p) c -> p eo c", p=P)
    for e in range(8):
        eng = nc.gpsimd if e % 2 == 0 else nc.sync
        eng.dma_start(wv_sb[:, e, :], wv_r[:, e, :])
    wo_sb = const.tile([P, 4, E], F16)
    wo_r = woT.rearrange("(co p) o -> p co o", p=P)
    for co in range(4):
        nc.gpsimd.dma_start(wo_sb[:, co, :], wo_r[:, co, :])

    # Causal mask for diagonal 128-blocks: mask[p, u] = 1 if u >= p else 0
    # (p = key within block, u = query within block).
    tmpp = tc.tile_pool(name="tmpf", bufs=1)
    tmp = tmpp.__enter__()
    mask_f = tmp.tile([P, P], F32)
    nc.gpsimd.memset(mask_f[:], 1.0)
    nc.gpsimd.affine_select(
        out=mask_f[:],
        in_=mask_f[:],
        compare_op=mybir.AluOpType.is_ge,
        fill=0.0,
        base=0,
        pattern=[[1, P]],        # + u
        channel_multiplier=-1,   # - p   => keep where u - p >= 0
    )
    mask = const.tile([P, P], F16)
    nc.vector.tensor_copy(mask[:], mask_f[:])

    # ---- persistent tensors ----
    # kT holds k-blocks 4..15 (cols 512:2048); blocks 0..3 live in kT0/the
    # per-iteration copy, see below.
    kT_sb = kvp.tile([P, 4, S], F16)             # [p, c, s]; ch = c*128+p
    v_sb = kvp.tile([P, NJ_ALL, GH, HD + 1], F16)  # [s%128, j, h, d(+ones)]
    # Dedicated window-0 buffers written by the pipelined next-iteration QKV.
    kT0 = kvp.tile([P, 4, 512], F16)
    v0 = kvp.tile([P, 4, GH, HD + 1], F16)
    qT0 = kvp.tile([P, 4, 512], F16)
    xs0 = kvp.tile([P, 8, 512], F16)
    xs0_8 = kvp.tile([P, 8, 512], F8)
    xs1 = kvp.tile([P, 8, 512], F16)
    xs1_8 = kvp.tile([P, 8, 512], F8)
    ctxT3 = kvp.tile([P, 4, 512], F16)
    # First bench-loop iteration reads ctxT3 before it is ever written (the
    # pipelined qb3 outproj of a nonexistent previous iteration) — zero it.
    nc.gpsimd.memset(ctxT3[:], 0.0)

    ones_f = tmp.tile([P, NJ_ALL * GH], F32)
    nc.vector.memset(ones_f[:], 1.0)
    nc.vector.tensor_copy(
        v_sb[:, :, :, HD],
        ones_f[:].rearrange("p (j h) -> p j h", j=NJ_ALL),
    )
    nc.vector.tensor_copy(
        v0[:, :, :, HD],
        ones_f[:, 0:4 * GH].rearrange("p (j h) -> p j h", j=4),
    )
    tmpp.__exit__(None, None, None)

    xTr = xT.rearrange("(eo p) s -> p eo s", p=P)
    xTr8 = xT8.rearrange("(eo p) s -> p eo s", p=P)

    if not has_bias:
        bqk_sb = bv_sb = None

    st = dict(nc=nc, tc=tc, xTr=xTr, out=out, wqk_sb=wqk_sb, wv_sb=wv_sb,
              wo_sb=wo_sb, bqk_sb=bqk_sb, bv_sb=bv_sb, mask=mask,
              kT_sb=kT_sb, v_sb=v_sb, kT0=kT0, v0=v0, qT0=qT0, xs0=xs0,
              xs0_8=xs0_8, xs1=xs1, xs1_8=xs1_8, xTr8=xTr8, ctxT3=ctxT3, qp=qp, xp=xp, xp8=xp8, cxp=cxp, ep=ep, osb=osb, bp=bp,
              ps_s=ps_s, ps_m=ps_m, xs_by_qb={})

    # ---- prologue: window-0 QKV for the first iteration ----
    nc.sync.dma_start(xs0[:], xTr[:, :, 0:512])
    nc.sync.dma_start(xs0_8[:], xTr8[:, :, 0:512])
    nc.sync.dma_start(xs1[:], xTr[:, :, 512:1024])
    nc.sync.dma_start(xs1_8[:], xTr8[:, :, 512:1024])
    st["xs_by_qb"][0] = (xs0, xs0_8)
    st["xs_by_qb"][1] = (xs1, xs1_8)
    for chain in _qkv_strip_chains(st, 0):
        chain()

    if bench_iters:
        # 2x unrolled loop: one all-engine back-edge barrier per two
        # iterations. The first body defers its qb3 out-projection into the
        # second body's window-0 fillers (no barrier between the two), so
        # only the second body's out-projection tail sits before the barrier.
        if bench_iters % 2 == 0:
            loop_cm = tc.For_i(0, bench_iters // 2, 1,
                               hint_engines=(mybir.EngineType.PE,
                                             mybir.EngineType.DVE,
                                             mybir.EngineType.Activation,
                                             mybir.EngineType.Pool,
                                             mybir.EngineType.SP))
            with loop_cm:
                _emit_body(st, pipelined=True, defer_out3=True)
                _emit_body(st, pipelined=True, take_deferred=True)
        else:
            loop_cm = tc.For_i(0, bench_iters, 1,
                               hint_engines=(mybir.EngineType.PE,
                                             mybir.EngineType.DVE,
                                             mybir.EngineType.Activation,
                                             mybir.EngineType.Pool,
                                             mybir.EngineType.SP))
            with loop_cm:
                _emit_body(st, pipelined=True)
    else:
        _emit_body(st, pipelined=False)

    for p in reversed(ctxmgr):
        p.__exit__(None, None, None)


def _publish_qb0(st):
    """Copy kT0/v0 (written during the previous iteration's last window)
    into the main kT/v banks so windows 1..3 read uniform addresses."""
    nc = st["nc"]

    def go():
        nc.vector.tensor_copy(st["kT_sb"][:, :, 0:512], st["kT0"][:])
        nc.vector.tensor_copy(st["v_sb"][:, 0:4, :, :], st["v0"][:])
    yield go


def _prefetch_x(st, qb):
    """DMA the x strip for window qb one window before its chains run.
    Strips 0/1 live in dedicated tiles (they wrap the back edge)."""
    nc = st["nc"]
    s0 = qb * 512
    if qb == 0:
        xs, xs8 = st["xs0"], st["xs0_8"]
    elif qb == 1:
        xs, xs8 = st["xs1"], st["xs1_8"]
    else:
        xs = st["xp"].tile([P, 8, 512], F16)
        xs8 = st["xp8"].tile([P, 8, 512], F8)
    st["xs_by_qb"][qb] = (xs, xs8)

    def go():
        nc.sync.dma_start(xs[:], st["xTr"][:, :, s0:s0 + 512])
        nc.sync.dma_start(xs8[:], st["xTr8"][:, :, s0:s0 + 512])
    yield go


def _emit_body(st, pipelined, defer_out3=False, take_deferred=False):
    """Software-pipelined emission: attention(qb) is the backbone; PE-only
    work — qkv(qb+1) chains and outproj(qb-1) chains — is spliced between
    individual j-iterations so the in-order PE stream always has independent
    matmuls to chew on while it waits for exp results.  In pipelined (bench
    loop) mode the qb3 outproj and the next iteration's qb0 qkv wrap around
    the loop back-edge."""
    ctx = [None, None, None, st["ctxT3"]]

    fillers = [_publish_qb0(st), _prefetch_x(st, 2), _qkv_strip_chains(st, 1)]
    nf0 = 15
    if take_deferred:
        # previous body in the unrolled pair deferred its qb3 outproj here;
        # no barrier sits between the two bodies, so this overlaps window 0.
        fillers.append(_outproj_chains(st, st["ctxT3"], 3))
        nf0 += 8
    ctx[0] = _attn(st, 0, fillers=_roundrobin(fillers), n_fillers=nf0)

    for qb in (1, 2):
        fillers = [_prefetch_x(st, (qb + 2) % 4),
                   _qkv_strip_chains(st, qb + 1),
                   _outproj_chains(st, ctx[qb - 1], qb - 1)]
        ctx[qb] = _attn(st, qb, fillers=_roundrobin(fillers), n_fillers=22)

    fillers = [_outproj_chains(st, ctx[2], 2)]
    nf3 = 8
    if pipelined:
        fillers.insert(0, _qkv_strip_chains(st, 0))
        fillers.insert(0, _prefetch_x(st, 1))
        nf3 += 14
    _attn(st, 3, fillers=_roundrobin(fillers), n_fillers=nf3)
    if not defer_out3:
        for chain in _outproj_chains(st, st["ctxT3"], 3):
            chain()


def _roundrobin(gens):
    gens = list(gens)
    while gens:
        g = gens.pop(0)
        try:
            yield next(g)
            gens.append(g)
        except StopIteration:
            pass


def _qkv_strip_chains(st, qb):
    """Yield one callable per accumulation chain (8 matmuls + a drain op).
    qb == 0 targets the dedicated qT0/kT0/v0 buffers (next-iteration or
    prologue window 0); other qbs target the rotating qT pool and main kT/v.
    """
    nc = st["nc"]
    wqk_sb, wv_sb = st["wqk_sb"], st["wv_sb"]
    bqk_sb = st["bqk_sb"]
    s0 = qb * 512
    state = {}

    def load_x():
        state["xs"], state["xs8"] = st["xs_by_qb"][qb]

    if qb == 0:
        qT = st["qT0"]
    else:
        qT = st["qp"].tile([P, 4, 512], F16, tag="qT", name=f"qT{qb % 2}")
    st.setdefault("qT_by_qb", {})[qb] = qT

    yield load_x

    def qk_chain(cb):
        xs8 = state["xs8"]
        pq = st["ps_m"].tile([P, 512], F32, tag="m")
        for e4 in range(4):
            nc.tensor.matmul(
                pq[:],
                wqk_sb[:, e4, :, cb * P:(cb + 1) * P],
                xs8[:, 2 * e4:2 * e4 + 2, :],
                start=(e4 == 0), stop=(e4 == 3),
                perf_mode=mybir.MatmulPerfMode.DoubleRow,
            )
        if cb < 4:
            dest = qT[:, cb, :]
        elif qb == 0:
            dest = st["kT0"][:, cb - 4, :]
        else:
            dest = st["kT_sb"][:, cb - 4, s0:s0 + 512]
        # weights were pre-scaled x64 on the host to clear fp8 subnormals
        if bqk_sb is not None:
            nc.vector.tensor_scalar(dest, pq[:], 1.0 / 64,
                                    bqk_sb[:, cb:cb + 1],
                                    op0=mybir.AluOpType.mult,
                                    op1=mybir.AluOpType.add)
        else:
            nc.vector.tensor_scalar_mul(dest, pq[:], 1.0 / 64)

    def v_chain(sv):
        xs = state["xs"]
        pv = st["ps_m"].tile([P, 512], F32, tag="m")
        for e in range(8):
            nc.tensor.matmul(
                pv[:],
                xs[:, e, sv * P:(sv + 1) * P],
                wv_sb[:, e, :],
                start=(e == 0), stop=(e == 7),
            )
        if qb == 0:
            dest = st["v0"][:, sv, :, 0:HD]
        else:
            dest = st["v_sb"][:, s0 // P + sv, :, 0:HD]
        nc.vector.tensor_copy(dest, pv[:].rearrange("p (h d) -> p h d", h=GH))

    # k and v chains first: the next window's attention needs kT/v before qT
    for cb in (4, 5, 6, 7):
        yield (lambda cb=cb: qk_chain(cb))
    for sv in range(4):
        yield (lambda sv=sv: v_chain(sv))
    for cb in (0, 1, 2, 3):
        yield (lambda cb=cb: qk_chain(cb))


def _emit_pv(st, pv2, c, use0, j, w0, ex, nj):
    nc = st["nc"]
    for hp in range(2):
        src = (st["v0"][:, j, 2 * c + hp, :] if use0
               else st["v_sb"][:, j, 2 * c + hp, :])
        nc.tensor.matmul(
            pv2[hp][0:HD + 1, w0:512],
            src,
            ex[:, hp, w0:512],
            start=(j == 0), stop=(j == nj - 1),
        )


def _outproj_chains(st, ctxT, qb):
    nc = st["nc"]
    q0 = qb * 512
    for sb_i in range(4):
        for ob in range(2):
            def chain(sb_i=sb_i, ob=ob):
                po = st["ps_m"].tile([P, 512], F32, tag="m")
                for cc in range(4):
                    nc.tensor.matmul(
                        po[:],
                        ctxT[:, cc, sb_i * P:(sb_i + 1) * P],
                        st["wo_sb"][:, cc, ob * 512:(ob + 1) * 512],
                        start=(cc == 0), stop=(cc == 3),
                    )
                ot = st["osb"].tile([P, 512], F16)
                nc.vector.tensor_copy(ot[:], po[:])
                nc.sync.dma_start(
                    st["out"][q0 + sb_i * P:q0 + (sb_i + 1) * P,
                              ob * 512:(ob + 1) * 512],
                    ot[:],
                )
            yield chain


def _attn(st, qb, fillers=None, n_fillers=22):
    """Attention for q-window qb. Heads 2c (SBUF partitions 0-63) and 2c+1
    (64-127) are processed together. Returns the ctxT tile."""
    nc = st["nc"]
    mask = st["mask"]
    bv_sb = st["bv_sb"]
    qT = st["qT_by_qb"][qb]
    if qb == 3:
        ctxT = st["ctxT3"]
    else:
        ctxT = st["cxp"].tile([P, 4, 512], F16)
    nj = 4 * (qb + 1)
    fillers = iter(fillers) if fillers is not None else iter(())
    done = False
    n_iters = 4 * nj
    acc = 0.0
    per_iter = float(n_fillers) / n_iters   # spread fillers over the window

    def emit_fillers(force_all=False):
        nonlocal acc, done
        if done:
            return
        acc += per_iter
        while (acc >= 1.0 or force_all) and not done:
            acc -= 1.0
            try:
                next(fillers)()
            except StopIteration:
                done = True

    for c in range(4):
        pv2 = [st["ps_m"].tile([P, 512], F32, tag="m", name=f"pv{hp}")
               for hp in range(2)]
        pend = None   # software-pipeline: PV trails scores by one j
        for j in range(nj):
            t = j - 4 * qb
            # Diagonal blocks only need q-columns >= t*128 (causality).
            w0 = 0 if t < 0 else t * P
            sp = st["ps_s"].tile([P, 2, 512], F32)
            for hp in range(2):
                p0 = 64 * hp
                src = (st["kT0"][p0:p0 + 64, c, j * P:(j + 1) * P] if qb == 0
                       else st["kT_sb"][p0:p0 + 64, c, j * P:(j + 1) * P])
                nc.tensor.matmul(
                    sp[:, hp, w0:512],
                    src,
                    qT[p0:p0 + 64, c, w0:512],
                    start=True, stop=True,
                )
            ex = st["ep"].tile([P, 2, 512], F16)
            nc.scalar.activation(ex[:, :, w0:512], sp[:, :, w0:512], AF.Exp)
            if t >= 0:
                # mask multiply over the diagonal band [t*128, (t+1)*128)
                m0 = t * P
                nc.vector.tensor_mul(
                    ex[:, :, m0:m0 + P],
                    ex[:, :, m0:m0 + P],
                    mask[:, None, :].to_broadcast((P, 2, P)),
                )
            if pend is not None:
                _emit_pv(st, pv2, c, qb == 0, *pend, nj)
            pend = (j, w0, ex)
            emit_fillers()
        if pend is not None:
            _emit_pv(st, pv2, c, qb == 0, *pend, nj)
        # normalize: ctxT = pv[0:64] / pv[64] (+ v bias)
        for hp in range(2):
            p0 = 64 * hp
            pv_ps = pv2[hp]
            bc = st["bp"].tile([64, 512], F32)
            nc.vector.reciprocal(bc[0:1, :], pv_ps[HD:HD + 1, :])
            nc.gpsimd.partition_broadcast(bc[:], bc[0:1, :])
            nc.vector.tensor_mul(ctxT[p0:p0 + 64, c, :], pv_ps[0:HD, :], bc[:])
            if bv_sb is not None:
                nc.vector.tensor_scalar_add(
                    ctxT[p0:p0 + 64, c, :],
                    ctxT[p0:p0 + 64, c, :],
                    bv_sb[p0:p0 + 64, c:c + 1],
                )
    emit_fillers(force_all=True)
    return ctxT


def _build_program(bench_iters=0, has_bias=True):
    nc = bacc.Bacc("TRN2", target_bir_lowering=False, debug=False,
                   num_devices=NCORES)
    xT = nc.dram_tensor("xT", [E, S], F16, kind="ExternalInput").ap()
    xT8 = nc.dram_tensor("xT8", [E, S], F8, kind="ExternalInput").ap()
    wqkT = nc.dram_tensor("wqkT", [E, 2 * GC], F8, kind="ExternalInput").ap()
    wvT = nc.dram_tensor("wvT", [E, GC], F16, kind="ExternalInput").ap()
    woT = nc.dram_tensor("woT", [GC, E], F16, kind="ExternalInput").ap()
    bqk = nc.dram_tensor("bqk", [8, P], F32, kind="ExternalInput").ap()
    bv = nc.dram_tensor("bv", [4, P], F32, kind="ExternalInput").ap()
    out = nc.dram_tensor("o", [S, E], F16, kind="ExternalOutput").ap()
    with tile.TileContext(nc) as tc:
        _emit(tc, nc, xT, xT8, wqkT, wvT, woT, bqk, bv, out,
              bench_iters=bench_iters, has_bias=has_bias)
    nc.compile()
    return nc


def _get_program(has_bias=True):
    if has_bias not in _program:
        _program[has_bias] = _build_program(has_bias=has_bias)
    return _program[has_bias]


def _make_in_maps(x, in_proj_w, in_proj_b, out_proj_w):
    scale = np.float32(1.0 / np.sqrt(HD))
    in_maps = []
    for c in range(NCORES):
        b, g = divmod(c, 2)
        lo, hi = g * GC, (g + 1) * GC
        wq = in_proj_w[lo:hi, :]
        wk = in_proj_w[E + lo:E + hi, :]
        wv = in_proj_w[2 * E + lo:2 * E + hi, :]
        wqkT = np.concatenate([wq.T * scale, wk.T], axis=1)
        bqk = np.concatenate([in_proj_b[lo:hi] * scale,
                              in_proj_b[E + lo:E + hi]]).reshape(8, P)
        bvv = in_proj_b[2 * E + lo:2 * E + hi].reshape(4, P)
        in_maps.append({
            "xT": np.ascontiguousarray(x[b].T).astype(NP16),
            "xT8": np.ascontiguousarray(x[b].T).astype(NP8),
            "wqkT": np.ascontiguousarray(wqkT * 64.0).astype(NP8),
            "wvT": np.ascontiguousarray(wv.T).astype(NP16),
            "woT": np.ascontiguousarray(out_proj_w[:, lo:hi].T).astype(NP16),
            "bqk": np.ascontiguousarray(bqk, dtype=np.float32),
            "bv": np.ascontiguousarray(bvv, dtype=np.float32),
        })
    return in_maps


def _combine(results, out_proj_b):
    out = np.empty((B, S, E), dtype=np.float32)
    for b in range(B):
        out[b] = (results[2 * b]["o"].astype(np.float32)
                  + results[2 * b + 1]["o"].astype(np.float32))
    out += np.asarray(out_proj_b, dtype=np.float32)[None, None, :]
    return out


def kernel(x, in_proj_w, in_proj_b, out_proj_w, out_proj_b, _trace=False):
    x = np.asarray(x, dtype=np.float32)
    in_proj_w = np.asarray(in_proj_w, dtype=np.float32)
    in_proj_b = np.asarray(in_proj_b, dtype=np.float32)
    out_proj_w = np.asarray(out_proj_w, dtype=np.float32)
    out_proj_b = np.asarray(out_proj_b, dtype=np.float32)
    assert x.shape == (B, S, E), x.shape

    has_bias = bool(np.any(in_proj_b))
    nc = _get_program(has_bias=has_bias)
    in_maps = _make_in_maps(x, in_proj_w, in_proj_b, out_proj_w)
    res = run_bass_kernel_spmd(nc, in_maps, core_ids=list(range(NCORES)),
                               trace=_trace)
    out = _combine(res.results, out_proj_b)
    if _trace:
        return out, res
    return out


# revision 7
# speedup vs baseline: 1.0414x; 1.0272x over previous
""" Causal multi-head attention (B=4, S=2048, E=1024, H=16) on 8 trn2 NeuronCores.

Sharding: core c = (batch b = c//2, head-group g = c%2). Each core computes
attention for one batch element and 8 of the 16 heads, plus the partial
out-projection restricted to its heads' columns. Host sums the two partial
outputs per batch element and adds the out-projection bias.

v2 over the baseline:
  - fp16 operands everywhere (PSUM accum stays fp32): halves SBUF/DMA/DVE
    cost and lifts the fp32r moving>=256 constraint so diagonal blocks trim
    to exactly the causal band.
  - cross-iteration software pipelining: the next iteration's first q-window
    QKV runs as PE filler during the last attention window, writing
    dedicated qT0/kT0/v0 buffers (no WAR hazard with the current iteration);
    cheap DVE copies publish kT0/v0 into the main kT/v banks during window 0.
    The qb3 out-projection likewise slides into the next iteration's window 0.
    This keeps the PE instruction stream gapless across the loop back-edge
    (PE pstate drops to 1.2 GHz after any idle and needs 3us busy to
    recover 2.4 GHz, so gaps cost ~2x on every matmul that follows).

Dataflow (all "transposed" so no on-device transposes):
  qT, kT  [ch, s]   from  W_chunk @ x.T        (lhsT = W.T tiles, rhs = x.T)
  v       [s, ch]   from  x @ Wv.T             (lhsT = x.T tiles, rhs = Wv.T)
  scoresT [k, q]    from  lhsT = kT, rhs = qT  (per head, K = 64)
  ctxT    [d, q]    from  lhsT = v (+ones col), rhs = exp(scoresT)
  outP    [s, o]    from  lhsT = ctxT, rhs = Wo.T
Softmax without max subtraction (scores bounded ~|2|); normalizer from a
ones-column in v; causal masking via one multiplicative [128,128] triangular
mask tile applied to the exp'd diagonal blocks.
"""

import sys

sys.path.insert(0, "/opt/trn_rl_repo")

import numpy as np

import concourse.bass as bass  # noqa: F401  (registers engine classes)
import concourse.mybir as mybir
import concourse.tile as tile
from concourse import bacc
from concourse.bass_utils import run_bass_kernel_spmd

F32 = mybir.dt.float32
F8 = mybir.dt.float8e4
NP8 = None  # set below
F16 = mybir.dt.float16
NP16 = np.float16
NP8 = mybir.dt.np(mybir.dt.float8e4)
AF = mybir.ActivationFunctionType

B, S, E = 4, 2048, 1024
H, HD = 16, 64
GH = 8                 # heads handled per core
GC = GH * HD           # 512 channels per head-group
P = 128
NCORES = 8
NJ_ALL = S // P        # 16 k-blocks of 128
QB = S // 512          # 4 q-windows of 512

_program = {}


def _emit(tc, nc, xT, xT8, wqkT, wvT, woT, bqk, bv, out, bench_iters=0,
          has_bias=True):
    ctxmgr = []

    def pool(**kw):
        p = tc.tile_pool(**kw)
        ctxmgr.append(p)
        return p.__enter__()

    const = pool(name="const", bufs=1)
    kvp = pool(name="kv", bufs=1)
    xp = pool(name="xs", bufs=2)
    xp8 = pool(name="xs8", bufs=2)
    qp = pool(name="qt", bufs=2)
    cxp = pool(name="ctx", bufs=2)
    ep = pool(name="expt", bufs=6)
    osb = pool(name="osb", bufs=2)
    bp = pool(name="bcast", bufs=2)
    ps_s = pool(name="ps_s", bufs=2, space="PSUM")
    ps_m = pool(name="ps_m", bufs=4, space="PSUM")

    # ---- constants ----
    # DMA order matters at startup: the first qkT matmuls need wqk + the
    # first x strip; wo is only needed later, so it is emitted last.
    bqk_sb = const.tile([P, 8], F32)
    nc.sync.dma_start(bqk_sb[:], bqk.rearrange("c p -> p c"))
    bv_sb = const.tile([P, 4], F32)
    nc.sync.dma_start(bv_sb[:], bv.rearrange("c p -> p c"))
    wqk_sb = const.tile([P, 4, 2, 2 * GC], F8)    # [p, e4, i2, ch]
    wqk_r = wqkT.rearrange("(eo p) c -> p eo c", p=P)
    for e in range(8):
        eng = nc.sync if e % 2 == 0 else nc.gpsimd
        eng.dma_start(wqk_sb[:, e // 2, e % 2, :], wqk_r[:, e, :])
    wv_sb = const.tile([P, 8, GC], F16)
    wv_r = wvT.rearrange("(eo p) c -> p eo c", p=P)
    for e in range(8):
        eng = nc.gpsimd if e % 2 == 0 else nc.sync
        eng.dma_start(wv_sb[:, e, :], wv_r[:, e, :])
    wo_sb = const.tile([P, 4, E], F16)
    wo_r = woT.rearrange("(co p) o -> p co o", p=P)
    for co in range(4):
        nc.gpsimd.dma_start(wo_sb[:, co, :], wo_r[:, co, :])

    # Causal mask for diagonal 128-blocks: mask[p, u] = 1 if u >= p else 0
    # (p = key within block, u = query within block).
    tmpp = tc.tile_pool(name="tmpf", bufs=1)
    tmp = tmpp.__enter__()
    mask_f = tmp.tile([P, P], F32)
    nc.gpsimd.memset(mask_f[:], 1.0)
    nc.gpsimd.affine_select(
        out=mask_f[:],
        in_=mask_f[:],
        compare_op=mybir.AluOpType.is_ge,
        fill=0.0,
        base=0,
        pattern=[[1, P]],        # + u
        channel_multiplier=-1,   # - p   => keep where u - p >= 0
    )
    mask = const.tile([P, P], F16)
    nc.vector.tensor_copy(mask[:], mask_f[:])

    # ---- persistent tensors ----
    # kT holds k-blocks 4..15 (cols 512:2048); blocks 0..3 live in kT0/the
    # per-iteration copy, see below.
    kT_sb = kvp.tile([P, 4, S], F16)             # [p, c, s]; ch = c*128+p
    v_sb = kvp.tile([P, NJ_ALL, GH, HD + 1], F16)  # [s%128, j, h, d(+ones)]
    # Dedicated window-0 buffers written by the pipelined next-iteration QKV.
    kT0 = kvp.tile([P, 4, 512], F16)
    v0 = kvp.tile([P, 4, GH, HD + 1], F16)
    qT0 = kvp.tile([P, 4, 512], F16)
    xs0 = kvp.tile([P, 8, 512], F16)
    xs0_8 = kvp.tile([P, 8, 512], F8)
    xs1 = kvp.tile([P, 8, 512], F16)
    xs1_8 = kvp.tile([P, 8, 512], F8)
    ctxT3 = kvp.tile([P, 4, 512], F16)
    # First bench-loop iteration reads ctxT3 before it is ever written (the
    # pipelined qb3 outproj of a nonexistent previous iteration) — zero it.
    nc.gpsimd.memset(ctxT3[:], 0.0)

    ones_f = tmp.tile([P, NJ_ALL * GH], F32)
    nc.vector.memset(ones_f[:], 1.0)
    nc.vector.tensor_copy(
        v_sb[:, :, :, HD],
        ones_f[:].rearrange("p (j h) -> p j h", j=NJ_ALL),
    )
    nc.vector.tensor_copy(
        v0[:, :, :, HD],
        ones_f[:, 0:4 * GH].rearrange("p (j h) -> p j h", j=4),
    )
    tmpp.__exit__(None, None, None)

    xTr = xT.rearrange("(eo p) s -> p eo s", p=P)
    xTr8 = xT8.rearrange("(eo p) s -> p eo s", p=P)

    if not has_bias:
        bqk_sb = bv_sb = None

    st = dict(nc=nc, tc=tc, xTr=xTr, out=out, wqk_sb=wqk_sb, wv_sb=wv_sb,
              wo_sb=wo_sb, bqk_sb=bqk_sb, bv_sb=bv_sb, mask=mask,
              kT_sb=kT_sb, v_sb=v_sb, kT0=kT0, v0=v0, qT0=qT0, xs0=xs0,
              xs0_8=xs0_8, xs1=xs1, xs1_8=xs1_8, xTr8=xTr8, ctxT3=ctxT3, qp=qp, xp=xp, xp8=xp8, cxp=cxp, ep=ep, osb=osb, bp=bp,
              ps_s=ps_s, ps_m=ps_m, xs_by_qb={})

    # ---- prologue: window-0 QKV for the first iteration ----
    nc.sync.dma_start(xs0[:], xTr[:, :, 0:512])
    nc.sync.dma_start(xs0_8[:], xTr8[:, :, 0:512])
    nc.sync.dma_start(xs1[:], xTr[:, :, 512:1024])
    nc.sync.dma_start(xs1_8[:], xTr8[:, :, 512:1024])
    st["xs_by_qb"][0] = (xs0, xs0_8)
    st["xs_by_qb"][1] = (xs1, xs1_8)
    for chain in _qkv_strip_chains(st, 0):
        chain()

    if bench_iters:
        # 2x unrolled loop: one all-engine back-edge barrier per two
        # iterations. The first body defers its qb3 out-projection into the
        # second body's window-0 fillers (no barrier between the two), so
        # only the second body's out-projection tail sits before the barrier.
        if bench_iters % 2 == 0:
            loop_cm = tc.For_i(0, bench_iters // 2, 1,
                               hint_engines=(mybir.EngineType.PE,
                                             mybir.EngineType.DVE,
                                             mybir.EngineType.Activation,
                                             mybir.EngineType.Pool,
                                             mybir.EngineType.SP))
            with loop_cm:
                _emit_body(st, pipelined=True, defer_out3=True)
                _emit_body(st, pipelined=True, take_deferred=True)
        else:
            loop_cm = tc.For_i(0, bench_iters, 1,
                               hint_engines=(mybir.EngineType.PE,
                                             mybir.EngineType.DVE,
                                             mybir.EngineType.Activation,
                                             mybir.EngineType.Pool,
                                             mybir.EngineType.SP))
            with loop_cm:
                _emit_body(st, pipelined=True)
    else:
        _emit_body(st, pipelined=False)

    for p in reversed(ctxmgr):
        p.__exit__(None, None, None)


def _publish_qb0(st):
    """Copy kT0/v0 (written during the previous iteration's last window)
    into the main kT/v banks so windows 1..3 read uniform addresses."""
    nc = st["nc"]

    def go():
        nc.vector.tensor_copy(st["kT_sb"][:, :, 0:512], st["kT0"][:])
        nc.vector.tensor_copy(st["v_sb"][:, 0:4, :, :], st["v0"][:])
    yield go


def _prefetch_x(st, qb):
    """DMA the x strip for window qb one window before its chains run.
    Strips 0/1 live in dedicated tiles (they wrap the back edge)."""
    nc = st["nc"]
    s0 = qb * 512
    if qb == 0:
        xs, xs8 = st["xs0"], st["xs0_8"]
    elif qb == 1:
        xs, xs8 = st["xs1"], st["xs1_8"]
    else:
        xs = st["xp"].tile([P, 8, 512], F16)
        xs8 = st["xp8"].tile([P, 8, 512], F8)
    st["xs_by_qb"][qb] = (xs, xs8)

    def go():
        nc.sync.dma_start(xs[:], st["xTr"][:, :, s0:s0 + 512])
        nc.sync.dma_start(xs8[:], st["xTr8"][:, :, s0:s0 + 512])
    yield go


def _emit_body(st, pipelined, defer_out3=False, take_deferred=False):
    """Software-pipelined emission: attention(qb) is the backbone; PE-only
    work — qkv(qb+1) chains and outproj(qb-1) chains — is spliced between
    individual j-iterations so the in-order PE stream always has independent
    matmuls to chew on while it waits for exp results.  In pipelined (bench
    loop) mode the qb3 outproj and the next iteration's qb0 qkv wrap around
    the loop back-edge."""
    ctx = [None, None, None, st["ctxT3"]]

    fillers = [_publish_qb0(st), _prefetch_x(st, 2), _qkv_strip_chains(st, 1)]
    nf0 = 15
    if take_deferred:
        # previous body in the unrolled pair deferred its qb3 outproj here;
        # no barrier sits between the two bodies, so this overlaps window 0.
        fillers.append(_outproj_chains(st, st["ctxT3"], 3))
        nf0 += 8
    ctx[0] = _attn(st, 0, fillers=_roundrobin(fillers), n_fillers=nf0)

    for qb in (1, 2):
        fillers = [_prefetch_x(st, (qb + 2) % 4),
                   _qkv_strip_chains(st, qb + 1),
                   _outproj_chains(st, ctx[qb - 1], qb - 1)]
        ctx[qb] = _attn(st, qb, fillers=_roundrobin(fillers), n_fillers=22)

    fillers = [_outproj_chains(st, ctx[2], 2)]
    nf3 = 8
    if pipelined:
        fillers.insert(0, _qkv_strip_chains(st, 0))
        fillers.insert(0, _prefetch_x(st, 1))
        nf3 += 14
    _attn(st, 3, fillers=_roundrobin(fillers), n_fillers=nf3)
    if not defer_out3:
        # In the loop (take_deferred body) this cluster sits right before the
        # all-engine back-edge barrier: issue its DMAs on two queues so the
        # barrier waits half the serial descriptor-generation time.
        for chain in _outproj_chains(st, st["ctxT3"], 3,
                                     dma_split=take_deferred):
            chain()


def _roundrobin(gens):
    gens = list(gens)
    while gens:
        g = gens.pop(0)
        try:
            yield next(g)
            gens.append(g)
        except StopIteration:
            pass


def _qkv_strip_chains(st, qb):
    """Yield one callable per accumulation chain (8 matmuls + a drain op).
    qb == 0 targets the dedicated qT0/kT0/v0 buffers (next-iteration or
    prologue window 0); other qbs target the rotating qT pool and main kT/v.
    """
    nc = st["nc"]
    wqk_sb, wv_sb = st["wqk_sb"], st["wv_sb"]
    bqk_sb = st["bqk_sb"]
    s0 = qb * 512
    state = {}

    def load_x():
        state["xs"], state["xs8"] = st["xs_by_qb"][qb]

    if qb == 0:
        qT = st["qT0"]
    else:
        qT = st["qp"].tile([P, 4, 512], F16, tag="qT", name=f"qT{qb % 2}")
    st.setdefault("qT_by_qb", {})[qb] = qT

    yield load_x

    def qk_chain(cb):
        xs8 = state["xs8"]
        pq = st["ps_m"].tile([P, 512], F32, tag="m")
        for e4 in range(4):
            nc.tensor.matmul(
                pq[:],
                wqk_sb[:, e4, :, cb * P:(cb + 1) * P],
                xs8[:, 2 * e4:2 * e4 + 2, :],
                start=(e4 == 0), stop=(e4 == 3),
                perf_mode=mybir.MatmulPerfMode.DoubleRow,
            )
        if cb < 4:
            dest = qT[:, cb, :]
        elif qb == 0:
            dest = st["kT0"][:, cb - 4, :]
        else:
            dest = st["kT_sb"][:, cb - 4, s0:s0 + 512]
        # weights were pre-scaled x64 on the host to clear fp8 subnormals
        if bqk_sb is not None:
            nc.vector.tensor_scalar(dest, pq[:], 1.0 / 64,
                                    bqk_sb[:, cb:cb + 1],
                                    op0=mybir.AluOpType.mult,
                                    op1=mybir.AluOpType.add)
        else:
            nc.vector.tensor_scalar_mul(dest, pq[:], 1.0 / 64)

    def v_chain(sv):
        xs = state["xs"]
        pv = st["ps_m"].tile([P, 512], F32, tag="m")
        for e in range(8):
            nc.tensor.matmul(
                pv[:],
                xs[:, e, sv * P:(sv + 1) * P],
                wv_sb[:, e, :],
                start=(e == 0), stop=(e == 7),
            )
        if qb == 0:
            dest = st["v0"][:, sv, :, 0:HD]
        else:
            dest = st["v_sb"][:, s0 // P + sv, :, 0:HD]
        nc.vector.tensor_copy(dest, pv[:].rearrange("p (h d) -> p h d", h=GH))

    # k and v chains first: the next window's attention needs kT/v before qT
    for cb in (4, 5, 6, 7):
        yield (lambda cb=cb: qk_chain(cb))
    for sv in range(4):
        yield (lambda sv=sv: v_chain(sv))
    for cb in (0, 1, 2, 3):
        yield (lambda cb=cb: qk_chain(cb))


def _emit_pv(st, pv2, c, use0, j, w0, ex, nj):
    nc = st["nc"]
    for hp in range(2):
        src = (st["v0"][:, j, 2 * c + hp, :] if use0
               else st["v_sb"][:, j, 2 * c + hp, :])
        nc.tensor.matmul(
            pv2[hp][0:HD + 1, w0:512],
            src,
            ex[:, hp, w0:512],
            start=(j == 0), stop=(j == nj - 1),
        )


def _outproj_chains(st, ctxT, qb, dma_split=False):
    nc = st["nc"]
    q0 = qb * 512
    for sb_i in range(4):
        for ob in range(2):
            def chain(sb_i=sb_i, ob=ob):
                po = st["ps_m"].tile([P, 512], F32, tag="m")
                for cc in range(4):
                    nc.tensor.matmul(
                        po[:],
                        ctxT[:, cc, sb_i * P:(sb_i + 1) * P],
                        st["wo_sb"][:, cc, ob * 512:(ob + 1) * 512],
                        start=(cc == 0), stop=(cc == 3),
                    )
                ot = st["osb"].tile([P, 512], F16)
                nc.vector.tensor_copy(ot[:], po[:])
                eng = nc.gpsimd if (dma_split and ob == 1) else nc.sync
                eng.dma_start(
                    st["out"][q0 + sb_i * P:q0 + (sb_i + 1) * P,
                              ob * 512:(ob + 1) * 512],
                    ot[:],
                )
            yield chain


def _attn(st, qb, fillers=None, n_fillers=22):
    """Attention for q-window qb. Heads 2c (SBUF partitions 0-63) and 2c+1
    (64-127) are processed together. Returns the ctxT tile."""
    nc = st["nc"]
    mask = st["mask"]
    bv_sb = st["bv_sb"]
    qT = st["qT_by_qb"][qb]
    if qb == 3:
        ctxT = st["ctxT3"]
    else:
        ctxT = st["cxp"].tile([P, 4, 512], F16)
    nj = 4 * (qb + 1)
    fillers = iter(fillers) if fillers is not None else iter(())
    done = False
    n_iters = 4 * nj
    acc = 0.0
    per_iter = float(n_fillers) / n_iters   # spread fillers over the window

    def emit_fillers(force_all=False):
        nonlocal acc, done
        if done:
            return
        acc += per_iter
        while (acc >= 1.0 or force_all) and not done:
            acc -= 1.0
            try:
                next(fillers)()
            except StopIteration:
                done = True

    for c in range(4):
        pv2 = [st["ps_m"].tile([P, 512], F32, tag="m", name=f"pv{hp}")
               for hp in range(2)]
        pend = None   # software-pipeline: PV trails scores by one j
        for j in range(nj):
            t = j - 4 * qb
            # Diagonal blocks only need q-columns >= t*128 (causality).
            w0 = 0 if t < 0 else t * P
            sp = st["ps_s"].tile([P, 2, 512], F32)
            for hp in range(2):
                p0 = 64 * hp
                src = (st["kT0"][p0:p0 + 64, c, j * P:(j + 1) * P] if qb == 0
                       else st["kT_sb"][p0:p0 + 64, c, j * P:(j + 1) * P])
                nc.tensor.matmul(
                    sp[:, hp, w0:512],
                    src,
                    qT[p0:p0 + 64, c, w0:512],
                    start=True, stop=True,
                )
            ex = st["ep"].tile([P, 2, 512], F16)
            nc.scalar.activation(ex[:, :, w0:512], sp[:, :, w0:512], AF.Exp)
            if t >= 0:
                # mask multiply over the diagonal band [t*128, (t+1)*128)
                m0 = t * P
                nc.vector.tensor_mul(
                    ex[:, :, m0:m0 + P],
                    ex[:, :, m0:m0 + P],
                    mask[:, None, :].to_broadcast((P, 2, P)),
                )
            if pend is not None:
                _emit_pv(st, pv2, c, qb == 0, *pend, nj)
            pend = (j, w0, ex)
            emit_fillers()
        if pend is not None:
            _emit_pv(st, pv2, c, qb == 0, *pend, nj)
        # normalize: ctxT = pv[0:64] / pv[64] (+ v bias)
        for hp in range(2):
            p0 = 64 * hp
            pv_ps = pv2[hp]
            bc = st["bp"].tile([64, 512], F32)
            nc.vector.reciprocal(bc[0:1, :], pv_ps[HD:HD + 1, :])
            nc.gpsimd.partition_broadcast(bc[:], bc[0:1, :])
            nc.vector.tensor_mul(ctxT[p0:p0 + 64, c, :], pv_ps[0:HD, :], bc[:])
            if bv_sb is not None:
                nc.vector.tensor_scalar_add(
                    ctxT[p0:p0 + 64, c, :],
                    ctxT[p0:p0 + 64, c, :],
                    bv_sb[p0:p0 + 64, c:c + 1],
                )
    emit_fillers(force_all=True)
    return ctxT


def _build_program(bench_iters=0, has_bias=True):
    nc = bacc.Bacc("TRN2", target_bir_lowering=False, debug=False,
                   num_devices=NCORES)
    xT = nc.dram_tensor("xT", [E, S], F16, kind="ExternalInput").ap()
    xT8 = nc.dram_tensor("xT8", [E, S], F8, kind="ExternalInput").ap()
    wqkT = nc.dram_tensor("wqkT", [E, 2 * GC], F8, kind="ExternalInput").ap()
    wvT = nc.dram_tensor("wvT", [E, GC], F16, kind="ExternalInput").ap()
    woT = nc.dram_tensor("woT", [GC, E], F16, kind="ExternalInput").ap()
    bqk = nc.dram_tensor("bqk", [8, P], F32, kind="ExternalInput").ap()
    bv = nc.dram_tensor("bv", [4, P], F32, kind="ExternalInput").ap()
    out = nc.dram_tensor("o", [S, E], F16, kind="ExternalOutput").ap()
    with tile.TileContext(nc) as tc:
        _emit(tc, nc, xT, xT8, wqkT, wvT, woT, bqk, bv, out,
              bench_iters=bench_iters, has_bias=has_bias)
    nc.compile()
    return nc


def _get_program(has_bias=True):
    if has_bias not in _program:
        _program[has_bias] = _build_program(has_bias=has_bias)
    return _program[has_bias]


def _make_in_maps(x, in_proj_w, in_proj_b, out_proj_w):
    scale = np.float32(1.0 / np.sqrt(HD))
    in_maps = []
    for c in range(NCORES):
        b, g = divmod(c, 2)
        lo, hi = g * GC, (g + 1) * GC
        wq = in_proj_w[lo:hi, :]
        wk = in_proj_w[E + lo:E + hi, :]
        wv = in_proj_w[2 * E + lo:2 * E + hi, :]
        wqkT = np.concatenate([wq.T * scale, wk.T], axis=1)
        bqk = np.concatenate([in_proj_b[lo:hi] * scale,
                              in_proj_b[E + lo:E + hi]]).reshape(8, P)
        bvv = in_proj_b[2 * E + lo:2 * E + hi].reshape(4, P)
        in_maps.append({
            "xT": np.ascontiguousarray(x[b].T).astype(NP16),
            "xT8": np.ascontiguousarray(x[b].T).astype(NP8),
            "wqkT": np.ascontiguousarray(wqkT * 64.0).astype(NP8),
            "wvT": np.ascontiguousarray(wv.T).astype(NP16),
            "woT": np.ascontiguousarray(out_proj_w[:, lo:hi].T).astype(NP16),
            "bqk": np.ascontiguousarray(bqk, dtype=np.float32),
            "bv": np.ascontiguousarray(bvv, dtype=np.float32),
        })
    return in_maps


def _combine(results, out_proj_b):
    out = np.empty((B, S, E), dtype=np.float32)
    for b in range(B):
        out[b] = (results[2 * b]["o"].astype(np.float32)
                  + results[2 * b + 1]["o"].astype(np.float32))
    out += np.asarray(out_proj_b, dtype=np.float32)[None, None, :]
    return out


def kernel(x, in_proj_w, in_proj_b, out_proj_w, out_proj_b, _trace=False):
    x = np.asarray(x, dtype=np.float32)
    in_proj_w = np.asarray(in_proj_w, dtype=np.float32)
    in_proj_b = np.asarray(in_proj_b, dtype=np.float32)
    out_proj_w = np.asarray(out_proj_w, dtype=np.float32)
    out_proj_b = np.asarray(out_proj_b, dtype=np.float32)
    assert x.shape == (B, S, E), x.shape

    has_bias = bool(np.any(in_proj_b))
    nc = _get_program(has_bias=has_bias)
    in_maps = _make_in_maps(x, in_proj_w, in_proj_b, out_proj_w)
    res = run_bass_kernel_spmd(nc, in_maps, core_ids=list(range(NCORES)),
                               trace=_trace)
    out = _combine(res.results, out_proj_b)
    if _trace:
        return out, res
    return out
